# revision 27
# baseline (speedup 1.0000x reference)
"""Single-head causal self-attention (B=4, S=4096, D=512) on 8 trn2 NeuronCores.

Sharding: 2 cores per batch element. Each core handles ALL queries of its
batch but only the even- or odd-indexed 128-row KV tiles (flash-decoding
style KV-parity split). This keeps the SPMD program identical across cores,
perfectly load-balances the causal triangle, and wastes no tiles.

Weight folding (host-side, f32): scores = x Wq^T Wk x^T = x G x^T with
G = Wq^T @ Wk, and the output projection is folded into the values:
Vt = V Wo^T = x (Wv^T Wo^T) = x H. The device then only computes
  R^T = (x G)^T           lhsT=G chunks, rhs=x^T          [b, q]
  Vt[k,e]                 lhsT=x^T local chunks, rhs=H    [k, e]
  S^T[k,q] = sum_b x^T[b,k]^T R[b,q]   (lhsT = x^T local) accum over b
  P^T      = exp(S^T * scale)          (ScalarE, PSUM->SBUF bf16)
  l[1,q]   = ones^T @ P^T              M=1 matmul, accum over k tiles in PSUM
  poT[e,q] = sum_k Vt[k,e]^T P^T[k,q]  accum over k tiles  (= final proj!)
This removes the K projection and the entire output projection from the PE,
and keeps the row-sum l off the slow engines entirely (PSUM accumulates).

Scheduling notes (from HW NTFF traces):
 - Score QK matmuls run fp8 DoubleRow. Isolated DR pairs between bf16 PV
   groups pay a ~210ns weight-load serialization per MM at each mode
   transition; clustering 3 score tiles per run amortizes it.
 - Inputs are pre-reshaped host-side so each tensor lands with ONE contiguous
   DMA (the Sync queue's ~0.6us per-dma issue cost was a startup bottleneck).
 - PSUM: 3 st banks + 4 attn banks + 1 l bank = 8 (full).
"""

import numpy as np
import ml_dtypes

import concourse.bass as bass
import concourse.tile as tile
from concourse import bacc, mybir
from concourse import bass_utils

B, S, D = 4, 4096, 512
TS = 128              # kv tile rows
QB = 512              # query block
NQB = S // QB         # 8 query blocks
NLT = S // TS // 2    # 16 local kv tiles per core
NDC = D // 128        # 4 chunks of d
SL = S // 2           # 2048 local kv rows
SCALE = 1.0 / float(np.sqrt(D))
BF16 = mybir.dt.bfloat16
F32 = mybir.dt.float32
FP8 = mybir.dt.float8e4
N_CORES = 8
USE_FP8_SCORES = True  # QK^T in fp8e4 DoubleRow (2 matmuls of K=256 per tile)
GROUP = 8              # score-tile cluster size (amortize DR<->bf16 transitions)


def build_kernel(repeat=1):
    nc = bacc.Bacc("TRN2", target_bir_lowering=False, debug=False)

    # all inputs pre-reshaped host-side to [128, ...] so each is ONE
    # contiguous DMA (chunk index folded into the free dim)
    xfl = nc.dram_tensor("xfl", [TS, NDC, SL], BF16, kind="ExternalInput").ap()
    h = nc.dram_tensor("h", [TS, NDC, D], BF16, kind="ExternalInput").ap()
    masks = nc.dram_tensor("masks", [TS, 2, QB], BF16, kind="ExternalInput").ap()
    if USE_FP8_SCORES:
        xfl8 = nc.dram_tensor("xfl8", [TS, NDC, SL], FP8, kind="ExternalInput").ap()
        xT8 = nc.dram_tensor("xT8", [TS, NDC, S], FP8, kind="ExternalInput").ap()
        g8 = nc.dram_tensor("g8", [TS, NDC, D], FP8, kind="ExternalInput").ap()
    else:
        xT = nc.dram_tensor("xT", [TS, NDC, S], BF16, kind="ExternalInput").ap()
        g = nc.dram_tensor("g", [TS, NDC, D], BF16, kind="ExternalInput").ap()
    poT = nc.dram_tensor("poT", [D, S], BF16, kind="ExternalOutput").ap()
    lrow = nc.dram_tensor("lrow", [1, S], F32, kind="ExternalOutput").ap()

    with tile.TileContext(nc) as tc:
        with tc.tile_pool(name="persist", bufs=1) as P:
            xfl_sb = P.tile([TS, NDC, SL], BF16, tag="xfl", name="xfl")
            h_sb = P.tile([TS, NDC, D], BF16, tag="h", name="h")
            # DMA issue order = first-consumption order: Vt-proj needs xfl+h
            # first (xfl cb-split so the first matmuls start early), then
            # R-proj needs g8+xT8, then scores need xfl8.
            # interleave transfers so each consumer's data lands just-in-time:
            # Vt-proj kt-tiles consume xfl cb-blocks in order, R-proj needs
            # g8 + xT8 (chunk pairs), scores need xfl8 last.
            CB = SL // QB
            if USE_FP8_SCORES:
                g8_sb = P.tile([TS, NDC, D], FP8, tag="g8", name="g8")
                xT8_sb = P.tile([TS, NDC, S], FP8, tag="xT8", name="xT8")
            else:
                xT_sb = P.tile([TS, NDC, S], BF16, tag="xT", name="xT")
                g_sb = P.tile([TS, NDC, D], BF16, tag="g", name="g")
            mask_sb = P.tile([TS, 2, QB], BF16, tag="mask", name="mask")

            nc.sync.dma_start(xfl_sb[:, :, 0:QB], xfl[:, :, 0:QB])
            nc.sync.dma_start(h_sb[:], h[:])
            for cb in range(1, CB):
                nc.sync.dma_start(xfl_sb[:, :, cb * QB:(cb + 1) * QB],
                                  xfl[:, :, cb * QB:(cb + 1) * QB])
            if USE_FP8_SCORES:
                nc.sync.dma_start(g8_sb[:], g8[:])
                nc.sync.dma_start(xT8_sb[:, 0:2, :], xT8[:, 0:2, :])
                nc.sync.dma_start(xT8_sb[:, 2:4, :], xT8[:, 2:4, :])
            else:
                nc.sync.dma_start(g_sb[:], g[:])
                nc.sync.dma_start(xT_sb[:, 0:2, :], xT[:, 0:2, :])
                nc.sync.dma_start(xT_sb[:, 2:4, :], xT[:, 2:4, :])
            nc.sync.dma_start(mask_sb[:], masks[:])
            if USE_FP8_SCORES:
                xfl8_sb = P.tile([TS, NDC, SL], FP8, tag="xfl8", name="xfl8")
                nc.sync.dma_start(xfl8_sb[:], xfl8[:])

            if USE_FP8_SCORES:
                R8_sb = P.tile([TS, NDC, S], FP8, tag="R8", name="R8")
            else:
                R_sb = P.tile([TS, NDC, S], BF16, tag="R", name="R")
            Vt_sb = [P.tile([TS, D], BF16, tag=f"Vt{kt}", name=f"Vt{kt}") for kt in range(NLT)]
            # ones column for the l row-sum matmul (l = ones^T @ P^T, M=1).
            # fp8 pair layout: the l matmuls run DoubleRow over two kv tiles
            # at once (l tolerates fp8: it's a 2048-term sum, errors average).
            ones8_sb = P.tile([TS, 2, 16], FP8, tag="ones8", name="ones8")
            nc.vector.memset(ones8_sb[:], 1.0)
            # per-qb row sums land here; one lrow DMA at the end
            lall_sb = P.tile([1, S], F32, tag="lall", name="lall")

            from contextlib import ExitStack
            with ExitStack() as rep_ctx:
                if repeat > 1:
                    # large body (>256 insts/engine): arm the branch prefetcher
                    # so the back-edge I$-hits instead of a ~4us IRAM refetch
                    rep_ctx.enter_context(tc.For_i(
                        0, repeat, 1,
                        hint_engines=(mybir.EngineType.PE, mybir.EngineType.Activation,
                                      mybir.EngineType.DVE, mybir.EngineType.SP)))
                # ---- projections ----
                with tc.tile_pool(name="proj_ps", bufs=4, space="PSUM") as PP:
                    # Vt[k,e]: lhsT = xfl[c, k-chunk], rhs = H[c, :]
                    for kt in range(NLT):
                        ps = PP.tile([TS, D], F32, tag="ps", name="ps_v")
                        for e in range(NDC):
                            nc.tensor.matmul(
                                ps[:], xfl_sb[:, e, kt * TS:(kt + 1) * TS], h_sb[:, e, :],
                                start=(e == 0), stop=(e == NDC - 1))
                        if kt % 2 == 0:
                            nc.vector.tensor_copy(Vt_sb[kt][:], ps[:])
                        else:
                            nc.scalar.activation(Vt_sb[kt][:], ps[:],
                                                 mybir.ActivationFunctionType.Copy)
                # R[b,q] = (xG)^T: lhsT = G[a, b-chunk], rhs = xT[a, colblock].
                # 8 parallel accumulation chains (one PSUM bank per colblock)
                # so each stationary G chunk is loaded once and streams 8
                # colblocks back-to-back.
                with tc.tile_pool(name="rproj_ps", bufs=1, space="PSUM") as RP:
                    CBORD = list(range(S // QB))
                    for dc in range(NDC):
                        pss = {cb: RP.tile([TS, QB], F32, tag=f"ps{cb}", name=f"ps_p{cb}")
                               for cb in CBORD}
                        if USE_FP8_SCORES:
                            for g2 in range(2):
                                for cb in CBORD:
                                    nc.tensor.matmul(
                                        pss[cb][:],
                                        g8_sb[:, 2 * g2:2 * g2 + 2, dc * TS:(dc + 1) * TS],
                                        xT8_sb[:, 2 * g2:2 * g2 + 2, cb * QB:(cb + 1) * QB],
                                        start=(g2 == 0), stop=(g2 == 1),
                                        perf_mode=mybir.MatmulPerfMode.DoubleRow)
                        else:
                            for e in range(NDC):
                                for cb in CBORD:
                                    nc.tensor.matmul(
                                        pss[cb][:], g_sb[:, e, dc * TS:(dc + 1) * TS],
                                        xT_sb[:, e, cb * QB:(cb + 1) * QB],
                                        start=(e == 0), stop=(e == NDC - 1))
                        for cb in CBORD:
                            if USE_FP8_SCORES:
                                dst = R8_sb[:, dc, cb * QB:(cb + 1) * QB]
                            else:
                                dst = R_sb[:, dc, cb * QB:(cb + 1) * QB]
                            # split PSUM drains across DVE and ScalarE: the
                            # R-proj phase is copy-bound on a single engine
                            if cb % 2 == 0:
                                nc.vector.tensor_copy(dst, pss[cb][:])
                            else:
                                nc.scalar.activation(dst, pss[cb][:],
                                                     mybir.ActivationFunctionType.Copy)

                # ---- attention (directly in output space) ----
                # Steps (qb j, kv tile lt) are processed in clusters of GROUP:
                # all scores of a cluster back-to-back (keeps the DR matmuls
                # contiguous), then their l/PV groups (bf16 contiguous).
                with tc.tile_pool(name="st_ps", bufs=4, space="PSUM") as STP, \
                     tc.tile_pool(name="attn_ps", bufs=1, space="PSUM") as ATP, \
                     tc.tile_pool(name="p_sb", bufs=8) as PSB, \
                     tc.tile_pool(name="p8_sb", bufs=12) as P8B, \
                     tc.tile_pool(name="o_sb", bufs=3) as OSB:
                    steps = [(j, lt) for j in range(NQB) for lt in range(2 * j + 2)]
                    nsteps = len(steps)
                    p_t = {}
                    p8_t = {}
                    attn_ps = None
                    l_ps = None

                    def step_cs(j, lt):
                        # upper diagonal tile (lt == 2j+1): q cols [0, QB/2)
                        # are causally dead for BOTH kv parities -> half width
                        if lt == 2 * j + 1:
                            return slice(QB // 2, QB)
                        return slice(0, QB)

                    def issue_score(s):
                        j, lt = steps[s]
                        cs = step_cs(j, lt)
                        qcol = slice(j * QB + cs.start, (j + 1) * QB)
                        st = STP.tile([TS, QB], F32, tag="st", name="st")
                        if USE_FP8_SCORES:
                            for gg in range(2):
                                nc.tensor.matmul(
                                    st[:, cs],
                                    xfl8_sb[:, 2 * gg:2 * gg + 2, lt * TS:(lt + 1) * TS],
                                    R8_sb[:, 2 * gg:2 * gg + 2, qcol],
                                    start=(gg == 0), stop=(gg == 1),
                                    perf_mode=mybir.MatmulPerfMode.DoubleRow)
                        else:
                            for dc in range(NDC):
                                nc.tensor.matmul(
                                    st[:, cs], xfl_sb[:, dc, lt * TS:(lt + 1) * TS],
                                    R_sb[:, dc, qcol],
                                    start=(dc == 0), stop=(dc == NDC - 1))
                        p = PSB.tile([TS, QB], BF16, tag="p", name="p")
                        nc.scalar.activation(
                            p[:, cs], st[:, cs], mybir.ActivationFunctionType.Exp, scale=SCALE)
                        if lt >= 2 * j:
                            nc.vector.tensor_mul(p[:, cs], p[:, cs],
                                                 mask_sb[:, lt - 2 * j, cs])
                        p_t[s] = p
                        # fp8 shadow copy (pair layout) feeding the DoubleRow
                        # l row-sum matmuls
                        pi = lt // 2
                        if lt % 2 == 0:
                            p8 = P8B.tile([TS, 2, QB], FP8, tag="p8", name="p8")
                            p8_t[(j, pi)] = p8
                        else:
                            p8 = p8_t[(j, pi)]
                        if lt == 2 * j + 1:
                            # diag-upper slot: zero its causally-dead half
                            nc.vector.memset(p8[:, 1, 0:QB // 2], 0.0)
                        nc.vector.tensor_copy(p8[:, lt % 2, cs], p[:, cs])

                    def issue_pv(s):
                        nonlocal attn_ps, l_ps
                        j, lt = steps[s]
                        nlt = 2 * j + 2
                        cs = step_cs(j, lt)
                        qcol = slice(j * QB, (j + 1) * QB)
                        if lt == 0:
                            attn_ps = [ATP.tile([TS, QB], F32, tag=f"attn{dc}", name=f"attn{dc}")
                                       for dc in range(NDC)]
                        p = p_t.pop(s)
                        for dc in range(NDC):
                            nc.tensor.matmul(
                                attn_ps[dc][:, cs], Vt_sb[lt][:, dc * TS:(dc + 1) * TS],
                                p[:, cs],
                                start=(lt == 0), stop=(lt == nlt - 1))
                        if lt == nlt - 1:
                            # l row-sums batched contiguously at qb end: M=1
                            # fp8 DoubleRow matmuls, two kv tiles per pass,
                            # accumulating in PSUM (row 0 of a transiently-
                            # borrowed st-pool bank).
                            l_ps = STP.tile([TS, QB], F32, tag="st", name="st_l")
                            npair = nlt // 2
                            for pi in range(npair):
                                p8 = p8_t.pop((j, pi))
                                nc.tensor.matmul(
                                    l_ps[0:1, :], ones8_sb[:, :, 0:1], p8[:],
                                    start=(pi == 0), stop=(pi == npair - 1),
                                    perf_mode=mybir.MatmulPerfMode.DoubleRow)
                            nc.scalar.activation(lall_sb[0:1, qcol], l_ps[0:1, :],
                                                 mybir.ActivationFunctionType.Copy)
                            for dc in range(NDC):
                                po_sb = OSB.tile([TS, QB], BF16, tag=f"po_sb{dc}", name=f"po_sb{dc}")
                                for half in range(2):
                                    hs = slice(half * (QB // 2), (half + 1) * (QB // 2))
                                    # alternate engines to release PSUM banks sooner
                                    eng = nc.vector if (dc + half) % 2 == 0 else nc.scalar
                                    if eng is nc.scalar:
                                        nc.scalar.activation(po_sb[:, hs], attn_ps[dc][:, hs],
                                                             mybir.ActivationFunctionType.Copy)
                                    else:
                                        nc.vector.tensor_copy(po_sb[:, hs], attn_ps[dc][:, hs])
                                nc.sync.dma_start(poT[dc * TS:(dc + 1) * TS, qcol], po_sb[:])

                    s = 0
                    while s < nsteps:
                        g = min(GROUP, nsteps - s)
                        for k in range(g):
                            issue_score(s + k)
                        for k in range(g):
                            issue_pv(s + k)
                        s += g
                    nc.sync.dma_start(lrow[0:1, :], lall_sb[0:1, :])
    nc.compile()
    return nc


_cache = {}


def _make_masks(h):
    m = np.zeros((2 * TS, QB), dtype=np.float32)
    k_r = np.arange(TS)[:, None]
    q_r = np.arange(QB)[None, :]
    for c in range(2):
        m[c * TS:(c + 1) * TS] = (q_r >= 128 * (2 * c + h) + k_r)
    # [2*TS, QB] -> [TS, 2, QB] for a single DMA
    return np.stack([m[:TS], m[TS:]], axis=1).astype(ml_dtypes.bfloat16)


def _chunkfold(a, n):
    """[n*TS, C] -> [TS, n, C] with the chunk index as a middle dim."""
    rows, C = a.shape
    assert rows == n * TS
    return np.ascontiguousarray(a.reshape(n, TS, C).transpose(1, 0, 2))


def kernel(x, Wq, Wk, Wv, Wo, bo):
    bf = ml_dtypes.bfloat16
    x = np.asarray(x, dtype=np.float32)
    Wq, Wk, Wv, Wo, bo = (np.asarray(a, dtype=np.float32) for a in (Wq, Wk, Wv, Wo, bo))
    if "nc" not in _cache:
        _cache["nc"] = build_kernel()
    nc = _cache["nc"]

    # fold the projections: scores = x G x^T, Vt = x H (= V Wo^T)
    Gf = np.ascontiguousarray(Wq.T @ Wk)
    H = np.ascontiguousarray(Wv.T @ Wo.T).astype(np.float32)
    mask_h = [_make_masks(0), _make_masks(1)]

    # local kv columns for parity h: 128-col tiles with global tile index % 2 == h
    col_idx = {}
    for h in range(2):
        tiles = [np.arange(TS * (2 * lt + h), TS * (2 * lt + h) + TS) for lt in range(NLT)]
        col_idx[h] = np.concatenate(tiles)

    in_maps = []
    for core in range(N_CORES):
        b, h = core // 2, core % 2
        xTb = np.ascontiguousarray(x[b].T)                # [D, S] f32
        xflb = np.ascontiguousarray(xTb[:, col_idx[h]])   # [D, SL] f32
        im = {
            "xfl": _chunkfold(xflb.astype(bf), NDC),
            "h": _chunkfold(H.astype(bf), NDC),
            "masks": mask_h[h],
        }
        if USE_FP8_SCORES:
            f8 = ml_dtypes.float8_e4m3
            im["xfl8"] = _chunkfold(xflb, NDC).astype(f8)
            im["xT8"] = _chunkfold(xTb, NDC).astype(f8)
            im["g8"] = _chunkfold(Gf, NDC).astype(f8)
        else:
            im["xT"] = _chunkfold(xTb.astype(bf), NDC)
            im["g"] = _chunkfold(Gf.astype(bf), NDC)
        in_maps.append(im)

    global _last_in_maps
    _last_in_maps = in_maps
    res = bass_utils.run_bass_kernel_spmd(nc, in_maps, core_ids=list(range(N_CORES)))

    out = np.zeros((B, S, D), dtype=np.float32)
    for b in range(B):
        r0, r1 = res.results[2 * b], res.results[2 * b + 1]
        l = (r0["lrow"] + r1["lrow"]).reshape(1, S)
        poTs = r0["poT"].astype(np.float32) + r1["poT"].astype(np.float32)
        out[b] = (poTs / l).T + bo.astype(np.float32)
    return out
